# revision 33
# baseline (speedup 1.0000x reference)
"""Bahdanau-attention kernel for 8 Trainium2 NeuronCores (SPMD, batch-sharded).

Algorithm: scores[t,s] = sum_h v_h * tanh(D[h,t] + E[h,s]) via a sine expansion
tanh(x) ~= sum_k b_k sin(w_k x) (F=4 refit, rel err ~5e-3), factored through the
angle-addition formula into 2F PSUM-accumulating bf16 matmuls over sin/cos
features of D and E. The host pre-transposes enc/dec to (H, N), folds w_0/2pi
into W1/W2, and casts to bf16, so the device does one 1-pass bf16 matmul per
side for the base projection u; higher frequencies x = m_k*u - 1/8 are range-
reduced with the f32 magic-constant round into a delta-shifted frac window
rb in [-0.5, 0.5], r = rb + 1/8 in [-3/8, 5/8], chosen so BOTH features are
sin(+-2pi*rb + pi/4) with |arg| <= 3.93, inside the ACT Sin table's usable
domain (measured on HW) — no abs op anywhere. k=0 needs no reduction at all
(|2pi*u| < 3.55). v*b_k folds into the d-side features via an on-chip-built
broadcast tile (Pool bf16 tt; k<=1 on DVE to dodge the Pool wake-up stall).
The encoder padding mask is added as -1e30 via rank-1 bf16 matmuls so exp's
fused accum_out yields masked row sums directly; the decoder mask folds into
the 1/sum scale. Engine layout: PE u-mms+masks+scores, ACT u-copies+sins+exps
(Sin table preloaded under the input DMAs), DVE frac chains+folds+softmax,
Pool k>=2 folds. Inputs packed into 4 DMAs on the SP ring; outputs split
across the SP and ACT HWDGE rings.
"""
import os
import sys

import numpy as np

if "/opt/trn_rl_repo" not in sys.path:
    sys.path.insert(0, "/opt/trn_rl_repo")

S, T, B, H = 512, 256, 8, 128
F = 4
OMEGA = np.array([0.29902329, 0.91308255, 1.68546067, 2.74503157], dtype=np.float64)
BK = np.array([1.22273482, 0.33545347, 0.12777969, 0.03355399], dtype=np.float64)
MAGIC = float(1.5 * 2**23)
TWO_PI = float(2.0 * np.pi)
HALF_PI = float(0.5 * np.pi)
NEG_BIG = -1.0e30

_CACHE = {}
LAST_EXEC_NS = None


def _try_install_trace_hook():
    """Best-effort NTFF profile hook for axon (used only when tracing)."""
    try:
        import contextlib
        import ctypes
        import types

        if "antenv.axon_hooks" in sys.modules:
            return
        lib = ctypes.CDLL("/opt/axon/libaxon_pjrt.so")
        if not hasattr(lib, "axon_start_nrt_profile"):
            return
        lib.axon_start_nrt_profile.argtypes = [
            ctypes.POINTER(ctypes.c_int64),
            ctypes.c_size_t,
        ]
        lib.axon_start_nrt_profile.restype = ctypes.c_int64
        lib.axon_stop_nrt_profile.argtypes = [ctypes.c_char_p]
        lib.axon_stop_nrt_profile.restype = ctypes.c_int64

        @contextlib.contextmanager
        def _hook(output_dir, device_ids):
            import jax

            jax.devices()
            if device_ids:
                ids = (ctypes.c_int64 * len(device_ids))(*device_ids)
                rc = lib.axon_start_nrt_profile(ids, len(device_ids))
            else:
                rc = lib.axon_start_nrt_profile(None, 0)
            if rc != 0:
                raise RuntimeError(f"axon_start_nrt_profile rc={rc}")
            try:
                yield
            finally:
                n = lib.axon_stop_nrt_profile(str(output_dir).encode())
                if n < 0:
                    raise RuntimeError(f"axon_stop_nrt_profile rc={n}")

        mod = types.ModuleType("antenv.axon_hooks")
        _h = _hook

        def set_axon_ntff_profile_hook(h):
            pass

        def get_axon_ntff_profile_hook():
            return _h

        mod.set_axon_ntff_profile_hook = set_axon_ntff_profile_hook
        mod.get_axon_ntff_profile_hook = get_axon_ntff_profile_hook
        sys.modules["antenv.axon_hooks"] = mod
        import antenv

        antenv.axon_hooks = mod
    except Exception:
        pass


def _build():
    if "nc" in _CACHE:
        return _CACHE["nc"]
    import concourse.bacc as bacc
    import concourse.tile as tile
    import concourse.mybir as mybir

    F32 = mybir.dt.float32
    F32R = mybir.dt.float32r
    BF16 = mybir.dt.bfloat16
    AF = mybir.ActivationFunctionType
    AL = mybir.AluOpType

    M = [float(OMEGA[k] / OMEGA[0]) for k in range(F)]

    nc = bacc.Bacc("TRN2", target_bir_lowering=False, debug=False, num_devices=8)

    pkA_d = nc.dram_tensor("pkA", [H, T + 2 * H], BF16, kind="ExternalInput")
    pkB_d = nc.dram_tensor("pkB", [H, S], BF16, kind="ExternalInput")
    pv_d = nc.dram_tensor("pv", [H, 8], F32, kind="ExternalInput")
    em_d = nc.dram_tensor("em", [1, S], BF16, kind="ExternalInput")
    out_d = nc.dram_tensor("out", [T, S], F32, kind="ExternalOutput")

    DELTA = 0.125
    QUARTER_PI = float(0.25 * np.pi)

    with tile.TileContext(nc) as tc:
        with (
            tc.tile_pool(name="cst", bufs=1) as cst,
            tc.tile_pool(name="ps", bufs=1, space="PSUM") as psp,
        ):
            # ---- input DMAs: projections on SP queue, rest on ACT queue ----
            with nc.named_scope("dma_in"):
                pkA_sb = cst.tile([H, T + 2 * H], BF16)
                nc.sync.dma_start(pkA_sb[:], pkA_d[:])
                pkB_sb = cst.tile([H, S], BF16)
                nc.sync.dma_start(pkB_sb[:], pkB_d[:])
                pv_sb = cst.tile([H, 8], F32)
                nc.sync.dma_start(pv_sb[:], pv_d[:])
                em_sb = cst.tile([1, S], BF16)
                nc.sync.dma_start(em_sb[:], em_d[:])
            decT = pkA_sb[:, 0:T]
            Wd = pkA_sb[:, T:T + H]
            We = pkA_sb[:, T + H:T + 2 * H]

            ones1 = cst.tile([1, H], BF16)
            nc.vector.memset(ones1[:], 1.0)
            hp = cst.tile([H, 1], F32)
            nc.vector.memset(hp[:], HALF_PI)
            hq = cst.tile([H, 1], F32)
            nc.vector.memset(hq[:], QUARTER_PI)
            dmy = cst.tile([H, 1], F32)
            nc.vector.memset(dmy[:], 0.0)
            dmy_s = cst.tile([H, 1], F32)
            dmy_e = cst.tile([H, 1], F32)
            dmyb = cst.tile([H, 16], BF16)
            nc.vector.memset(dmyb[:], 1.0)
            nc.gpsimd.tensor_tensor(dmyb[:], dmyb[:], dmyb[:], AL.mult)

            # ---- vbt = v*b_k broadcast, built on-chip while DMAs stream ----
            vbt_sb = cst.tile([H, F * T], BF16)
            onesT = cst.tile([H, T], BF16)
            nc.vector.memset(onesT[:], 1.0)
            with nc.named_scope("vbt_build"):
                for k in range(F):
                    nc.vector.tensor_scalar_mul(vbt_sb[:, k * T:(k + 1) * T], onesT[:], pv_sb[:, k:k + 1])

            # ---- ACT Sin table preload hidden under the input DMAs ----
            with nc.named_scope("tbl_preload"):
                nc.scalar.activation(dmy_s[:], dmy[:], AF.Sin)

            # ---- base projections u = scal0 * (x @ W) in PSUM ----
            ud_ps = psp.tile([H, T], F32, name="ud")
            ue_ps = psp.tile([H, S], F32, name="ue")
            sc = [psp.tile([H, S], F32, name=f"sc{tb}") for tb in range(2)]
            ud_sb = cst.tile([H, T], F32)
            ue_sb = cst.tile([H, S], F32)
            # ---- feature tiles (k>=1 slices; k=0 reads u from PSUM) ----
            x_d = cst.tile([H, (F - 1) * T], F32)
            i_d = cst.tile([H, (F - 1) * T], F32)
            r_d = cst.tile([H, (F - 1) * T], BF16)
            x_e = cst.tile([H, (F - 1) * S], F32)
            i_e = cst.tile([H, (F - 1) * S], F32)
            r_e = cst.tile([H, (F - 1) * S], BF16)
            fSd = cst.tile([H, F * T], BF16)
            fCd = cst.tile([H, F * T], BF16)
            fSe = cst.tile([H, F * S], BF16)
            fCe = cst.tile([H, F * S], BF16)

            with nc.named_scope("u_mm"):
                nc.tensor.matmul(ud_ps[:], Wd, decT, start=True, stop=True)
                nc.tensor.matmul(ue_ps[:], We, pkB_sb[:], start=True, stop=True)
                # single PSUM reader per u tile; everything else reads SBUF
                nc.scalar.copy(ud_sb[:], ud_ps[:])
            with nc.named_scope("d0_acts"):
                nc.scalar.activation(fSd[:, 0:T], ud_sb[:], AF.Sin, scale=TWO_PI)
                nc.scalar.activation(fCd[:, 0:T], ud_sb[:], AF.Sin, bias=hp[:], scale=-TWO_PI)
            nc.scalar.copy(ue_sb[:], ue_ps[:])
            with nc.named_scope("mask_mm"):
                for tb in range(2):
                    nc.tensor.matmul(
                        sc[tb][:], ones1[:], em_sb[:],
                        start=True, stop=False, skip_group_check=True,
                    )

            def dsl(k):
                return slice(k * T, (k + 1) * T)

            def dxl(k):
                return slice((k - 1) * T, k * T)

            def esl(k):
                return slice(k * S, (k + 1) * S)

            def exl(k):
                return slice((k - 1) * S, k * S)

            # ---- PSUM consumers first: x' = m_k*u - delta for all k>=1 ----
            with nc.named_scope("xprime"):
                for k in range(1, F):
                    nc.vector.tensor_scalar(x_d[:, dxl(k)], ud_sb[:], M[k], -DELTA, AL.mult, AL.add)
                for k in range(1, F):
                    nc.vector.tensor_scalar(x_e[:, exl(k)], ue_sb[:], M[k], -DELTA, AL.mult, AL.add)

            def d_chain(k):
                # sin(2pi*m_k*u) and cos(2pi*m_k*u) via the delta-shifted frac
                # window: rb = x' - round(x'), x' = m_k*u - delta, so both
                # features are sin(+-2pi*rb + pi/4) with |arg| <= 3.93.
                with nc.named_scope(f"d{k}"):
                    if k == 0:
                        pass  # k0 activations emitted right after the ud copy
                    else:
                        nc.vector.tensor_scalar(i_d[:, dxl(k)], x_d[:, dxl(k)], MAGIC, MAGIC, AL.add, AL.subtract)
                        nc.vector.tensor_tensor(r_d[:, dxl(k)], x_d[:, dxl(k)], i_d[:, dxl(k)], AL.subtract)
                        nc.scalar.activation(fSd[:, dsl(k)], r_d[:, dxl(k)], AF.Sin, bias=hq[:], scale=TWO_PI)
                        nc.scalar.activation(fCd[:, dsl(k)], r_d[:, dxl(k)], AF.Sin, bias=hq[:], scale=-TWO_PI)
                    eng = nc.vector if k <= 1 else nc.gpsimd
                    eng.tensor_tensor(fSd[:, dsl(k)], fSd[:, dsl(k)], vbt_sb[:, dsl(k)], AL.mult)
                    eng.tensor_tensor(fCd[:, dsl(k)], fCd[:, dsl(k)], vbt_sb[:, dsl(k)], AL.mult)

            def e_chain(k):
                with nc.named_scope(f"e{k}"):
                    if k == 0:
                        nc.scalar.activation(fSe[:, esl(0)], ue_sb[:], AF.Sin, scale=TWO_PI)
                        nc.scalar.activation(fCe[:, esl(0)], ue_sb[:], AF.Sin, bias=hp[:], scale=-TWO_PI)
                    else:
                        nc.vector.tensor_scalar(i_e[:, exl(k)], x_e[:, exl(k)], MAGIC, MAGIC, AL.add, AL.subtract)
                        nc.vector.tensor_tensor(r_e[:, exl(k)], x_e[:, exl(k)], i_e[:, exl(k)], AL.subtract)
                        nc.scalar.activation(fSe[:, esl(k)], r_e[:, exl(k)], AF.Sin, bias=hq[:], scale=TWO_PI)
                        nc.scalar.activation(fCe[:, esl(k)], r_e[:, exl(k)], AF.Sin, bias=hq[:], scale=-TWO_PI)

            nc.gpsimd.tensor_tensor(dmyb[:], r_d[0:H, 0:16], r_d[0:H, 16:32], AL.mult)
            nc.gpsimd.tensor_tensor(dmyb[:], r_e[0:H, 512:528], r_e[0:H, 528:544], AL.mult)

            def scores(k):
                with nc.named_scope(f"sc{k}"):
                    for tb in range(2):
                        tsl = slice(k * T + tb * 128, k * T + (tb + 1) * 128)
                        nc.tensor.matmul(
                            sc[tb][:], fSd[:, tsl], fCe[:, esl(k)],
                            start=False, stop=False, skip_group_check=True,
                        )
                        nc.tensor.matmul(
                            sc[tb][:], fCd[:, tsl], fSe[:, esl(k)],
                            start=False, stop=(k == F - 1), skip_group_check=True,
                        )

            for k in range(F):
                d_chain(k)
                e_chain(k)
                scores(k)

            # ---- softmax (no max-shift; scores bounded) + masked scale ----
            for tb in range(2):
                with nc.named_scope(f"softmax_{tb}"):
                    ex = cst.tile([128, S], F32, name=f"ex{tb}")
                    rs = cst.tile([128, 1], F32, name=f"rs{tb}")
                    nc.scalar.activation(ex[:], sc[tb][:], AF.Exp, accum_out=rs[:])
                    ri = cst.tile([128, 1], F32, name=f"ri{tb}")
                    nc.vector.reciprocal(ri[:], rs[:])
                    fac = cst.tile([128, 1], F32, name=f"fac{tb}")
                    nc.vector.tensor_tensor(fac[:], ri[:], pv_sb[:, 4 + tb:5 + tb], AL.mult)
                    ot = cst.tile([128, S], F32, name=f"ot{tb}")
                    nc.vector.tensor_scalar_mul(ot[:], ex[:], fac[:])
                    nc.sync.dma_start(out_d[tb * 128:tb * 128 + 64, :], ot[0:64, :])
                    nc.scalar.dma_start(out_d[tb * 128 + 64:(tb + 1) * 128, :], ot[64:128, :])

    nc.compile()
    _CACHE["nc"] = nc
    return nc


def kernel(encoder_output, decoder_output, W1, W2, v, enc_lens, dec_lens):
    global LAST_EXEC_NS
    from concourse.bass_utils import run_bass_kernel_spmd
    import ml_dtypes

    enc = np.asarray(encoder_output, dtype=np.float32)
    dec = np.asarray(decoder_output, dtype=np.float32)
    W1 = np.asarray(W1, dtype=np.float32)
    W2 = np.asarray(W2, dtype=np.float32)
    v = np.asarray(v, dtype=np.float32)
    enc_lens = np.asarray(enc_lens)
    dec_lens = np.asarray(dec_lens)

    scal0 = np.float64(OMEGA[0] / (2.0 * np.pi))
    Wd = (W2.astype(np.float64) * scal0).astype(np.float32)   # (H, H) lhsT for u_d
    We = (W1.astype(np.float64) * scal0).astype(np.float32)
    vb = (v[:, None].astype(np.float64) * BK[None, :]).astype(np.float32)  # (H, F)

    in_maps = []
    for b in range(B):
        decT = np.ascontiguousarray(dec[:, b, :].T)           # (H, T)
        encT = np.ascontiguousarray(enc[:, b, :].T).astype(ml_dtypes.bfloat16)
        pkA = np.concatenate([decT, Wd, We], axis=1).astype(ml_dtypes.bfloat16)
        pv = np.zeros((H, 8), dtype=np.float32)
        pv[:, 0:F] = vb
        tidx = np.arange(T).reshape(2, 128)                   # dm[p, tb]
        pv[:, 4:6] = (tidx.T < int(dec_lens[b])).astype(np.float32)
        em = np.where(np.arange(S)[None, :] < int(enc_lens[b]), 0.0, NEG_BIG)
        in_maps.append(
            {
                "pkA": pkA,
                "pkB": encT,
                "pv": pv,
                "em": em.astype(ml_dtypes.bfloat16),
            }
        )

    trace = os.environ.get("KERNEL_TRACE", "0") == "1"
    if trace:
        _try_install_trace_hook()
    nc = _build()
    ncores = int(os.environ.get("KERNEL_CORES", str(B)))
    # warmup execution: loads the NEFF and brings the device out of its
    # idle clock state so the measured run sees a warm clock
    for _ in range(int(os.environ.get("KERNEL_WARMUP", "2"))):
        run_bass_kernel_spmd(nc, in_maps[:ncores], core_ids=list(range(ncores)), trace=False)
    res = run_bass_kernel_spmd(nc, in_maps[:ncores], core_ids=list(range(ncores)), trace=trace)
    if trace:
        LAST_EXEC_NS = res.exec_time_ns
        _CACHE["last_res"] = res

    out = np.zeros((T, B, S), dtype=np.float32)
    for b in range(ncores):
        out[:, b, :] = res.results[b]["out"]
    return out
